# revision 6
# baseline (speedup 1.0000x reference)
"""ChunkedAttention (nn_ChunkedAttention_43568148251092) Trainium2 kernel.

Full inputs q/k/v: [1, 4096, 16, 128] fp32. Shards the 16 heads across the
8 NeuronCores (2 heads per core, pure head parallelism - no collectives),
runs a Bass/Tile attention kernel per core, and reassembles the results.

Per-core pipeline (2 heads, S=4096 tokens, D=128), ACT-exp is the roofline
(2*S^2 exps at ~1.2GHz*128 lanes ~= 266us); everything else is organized to
hide under it:
  - Host stages layouts only: Q fed pre-transposed [H, 128(d), 4096(s)],
    K/V per-head contiguous [H, 4096, 128], output per-head contiguous.
  - K/V int8 quant-dequant per token exactly like the reference
    (trunc-toward-zero), via a 5-op DVE chain: y = kv*(1/scale);
    y2 = (y + c) - (kv>0)*2c with c = 0.499995; RNE int32 convert.
    This equals trunc() except on a ~1e-5-wide window per integer boundary.
  - Kint (exact in fp16) is PE-transposed to KT [d, s]; per-token
    kscale/sqrt(D) folded into the softmax exp via the ACT per-partition
    scale. Q cast to fp16 on DVE straight into QT layout (no transposes).
  - S^T[k,q] = KT_tile.T @ QT in PSUM fp32 (512-wide matmuls).
  - P'[k,q] = exp(kscale*S^T - 40) on ACT ([128,1024] tiles) -> bf16.
  - out[q, 0:128|denom] = sum_kt P'_kt.T @ [Vdq | ones] in PSUM; the
    ones-column gives the softmax denominator for free; DVE divides.
  - Software pipelining: QK matmuls for chunk s+1 are emitted before PV for
    chunk s (no PE head-of-line blocking); head h+1 preprocessing is woven
    into head h's main-loop slots; head 0's first QK quads are woven into
    its K-quant groups so ACT starts ~6us into the kernel.
"""

import math
import time

import numpy as np

import concourse.bass as bass
import concourse.mybir as mybir
import concourse.tile as tile
from concourse import bacc
from concourse.bass_utils import run_bass_kernel_spmd
from concourse.masks import make_identity

F32 = mybir.dt.float32
BF16 = mybir.dt.bfloat16
FP16 = mybir.dt.float16
I32 = mybir.dt.int32
AX = mybir.AxisListType.X
OP = mybir.AluOpType
EXP = mybir.ActivationFunctionType.Exp

_S = 4096
_H_TOTAL = 16
_D = 128
_N_CORES = 8
_H = _H_TOTAL // _N_CORES  # heads per core

_NC_CACHE = {}

# trunc-toward-zero shift constant (see module docstring)
_C = 0.499995


def _bcast3(ap2, n):
    """[128, J] AP -> [128, J, n] broadcast AP (inner stride 0)."""
    return bass.AP(tensor=ap2.tensor, offset=ap2.offset, ap=[*ap2.ap, [0, n]])


def _build_nc(S=_S, H=_H, D=_D, qc_cols=1024, c_bias=40.0,
              pp_bufs=52, psS_bufs=2, ld_bufs=6, tmp_bufs=2):
    assert D == 128 and S % 512 == 0 and qc_cols == 1024
    n_kt = S // 128       # 32 K tiles per head
    n_grp = S // 512      # 8 quant groups per head
    n_qc = S // qc_cols   # 4 q chunks per head
    qt_per_qc = qc_cols // 128
    n_slots = H * n_qc

    nc = bacc.Bacc("TRN2")
    qt_d = nc.dram_tensor("qt", [H, D, S], F32, kind="ExternalInput")
    k_d = nc.dram_tensor("k", [H, S, D], F32, kind="ExternalInput")
    v_d = nc.dram_tensor("v", [H, S, D], F32, kind="ExternalInput")
    o_d = nc.dram_tensor("o", [H, S, D], F32, kind="ExternalOutput")

    with tile.TileContext(nc) as tc:
        with (
            tc.tile_pool(name="const", bufs=1) as constp,
            tc.tile_pool(name="big", bufs=2) as bigp,
            tc.tile_pool(name="qld", bufs=2) as qldp,
            tc.tile_pool(name="ld", bufs=ld_bufs) as ldp,
            tc.tile_pool(name="tmp", bufs=tmp_bufs) as tmpp,
            tc.tile_pool(name="b16", bufs=4) as b16p,
            tc.tile_pool(name="small", bufs=8) as smallp,
            tc.tile_pool(name="pp", bufs=pp_bufs) as ppool,
            tc.tile_pool(name="outp", bufs=4) as outp,
            tc.tile_pool(name="psT", bufs=2, space="PSUM") as psT,
            tc.tile_pool(name="psS", bufs=psS_bufs, space="PSUM") as psS,
            tc.tile_pool(name="psO", bufs=2, space="PSUM") as psO,
        ):
            ident32 = constp.tile([128, 128], F32)
            make_identity(nc, ident32[:])
            ident16 = constp.tile([128, 128], FP16)
            nc.vector.tensor_copy(ident16[:], ident32[:])
            bias_t = constp.tile([128, 1], F32)
            nc.vector.memset(bias_t[:], -c_bias)

            heads = [None] * H

            def alloc_head(h):
                heads[h] = {
                    "KT": bigp.tile([128, S], FP16, tag="KT", name=f"KT{h}"),
                    "QT": bigp.tile([128, S], FP16, tag="QT", name=f"QT{h}"),
                    "Vext": bigp.tile([128, n_kt, 132], BF16, tag="Vext",
                                      name=f"Vext{h}"),
                    "ksc": bigp.tile([128, n_kt], F32, tag="ksc",
                                     name=f"ksc{h}"),
                    "pt": {},
                }

            def emit_vext_const(h):
                V = heads[h]["Vext"]
                nc.vector.memset(V[:, :, 128:132], 0.0)
                nc.vector.memset(V[:, :, 128:129], 1.0)

            def emit_qchunk(h, c):
                c0 = c * 1024
                qf = qldp.tile([128, 1024], F32, tag="qld")
                nc.sync.dma_start(out=qf[:], in_=qt_d[h, :, c0:c0 + 1024])
                nc.vector.tensor_copy(heads[h]["QT"][:, c0:c0 + 1024], qf[:])

            def quant_common(src_d, h, g):
                """Load + scale + trunc -> (i32 tile [128,4,128], sc [128,4])."""
                s0 = g * 512
                xf = ldp.tile([128, 4, 128], F32, tag="ld")
                nc.sync.dma_start(
                    out=xf[:],
                    in_=src_d[h, s0:s0 + 512, :].rearrange(
                        "(j p) d -> p j d", p=128))
                am = smallp.tile([128, 4], F32, tag="am")
                nc.vector.reduce_max(am[:], xf[:], axis=AX,
                                     apply_absolute_value=True)
                sc = smallp.tile([128, 4], F32, tag="sc")
                nc.vector.tensor_scalar(sc[:], am[:], 1e-8, 1.0 / 127.0,
                                        op0=OP.max, op1=OP.mult)
                rc = smallp.tile([128, 4], F32, tag="rc")
                nc.vector.reciprocal(rc[:], sc[:])
                y = tmpp.tile([128, 4, 128], F32, tag="t_y")
                nc.vector.tensor_tensor(y[:], xf[:], _bcast3(rc[:], 128),
                                        op=OP.mult)
                g2c = tmpp.tile([128, 4, 128], F32, tag="t_g")
                nc.vector.tensor_scalar(g2c[:], xf[:], 0.0, 2.0 * _C,
                                        op0=OP.is_gt, op1=OP.mult)
                nc.vector.scalar_tensor_tensor(y[:], y[:], _C, g2c[:],
                                               op0=OP.add, op1=OP.subtract)
                xi = tmpp.tile([128, 4, 128], I32, tag="t_i")
                nc.vector.tensor_copy(xi[:], y[:])
                return xi, sc

            def emit_kgroup(h, g):
                hd = heads[h]
                xi, sc = quant_common(k_d, h, g)
                nc.vector.tensor_scalar(hd["ksc"][:, 4 * g:4 * g + 4], sc[:],
                                        1.0 / math.sqrt(128.0), None,
                                        op0=OP.mult)
                kint = b16p.tile([128, 4, 128], FP16, tag="i16")
                nc.vector.tensor_copy(kint[:], xi[:])
                for j in range(4):
                    pst = psT.tile([128, 128], FP16, tag="pst")
                    nc.tensor.transpose(pst[:], kint[:, j, :], ident16[:])
                    kt_i = 4 * g + j
                    nc.vector.tensor_copy(
                        hd["KT"][:, kt_i * 128:(kt_i + 1) * 128], pst[:])

            def emit_vgroup(h, g):
                hd = heads[h]
                xi, sc = quant_common(v_d, h, g)
                vf = tmpp.tile([128, 4, 128], F32, tag="t_v")
                nc.vector.tensor_copy(vf[:], xi[:])
                nc.vector.tensor_tensor(
                    hd["Vext"][:, 4 * g:4 * g + 4, 0:128], vf[:],
                    _bcast3(sc[:], 128), op=OP.mult)

            def emit_qk_one(h, qc, kt):
                hd = heads[h]
                sps = psS.tile([128, qc_cols], F32, tag="sps")
                w = hd["KT"][:, kt * 128:(kt + 1) * 128]
                for half in range(2):
                    c0 = qc * qc_cols + half * 512
                    nc.tensor.matmul(sps[:, half * 512:(half + 1) * 512],
                                     w, hd["QT"][:, c0:c0 + 512],
                                     start=True, stop=True)
                pt = ppool.tile([128, qc_cols], BF16, tag="pp")
                nc.scalar.activation(pt[:], sps[:], EXP, bias=bias_t[:],
                                     scale=hd["ksc"][:, kt:kt + 1])
                hd["pt"][(qc, kt)] = pt

            def emit_qk(h, qc):
                for kt in range(n_kt):
                    emit_qk_one(h, qc, kt)

            def emit_pv(h, qc):
                hd = heads[h]
                for qt in range(qt_per_qc):
                    ops_ = psO.tile([128, 132], F32, tag="ops")
                    for kt in range(n_kt):
                        nc.tensor.matmul(
                            ops_[:],
                            hd["pt"][(qc, kt)][:, qt * 128:(qt + 1) * 128],
                            hd["Vext"][:, kt, :],
                            start=(kt == 0), stop=(kt == n_kt - 1))
                    rcp = smallp.tile([128, 1], F32, tag="rcp")
                    nc.vector.reciprocal(rcp[:], ops_[:, 128:129])
                    ot = outp.tile([128, 128], F32, tag="ot")
                    nc.vector.tensor_scalar(ot[:], ops_[:, 0:128], rcp[:],
                                            None, op0=OP.mult)
                    q0 = qc * qc_cols + qt * 128
                    nc.sync.dma_start(out=o_d[h, q0:q0 + 128, :], in_=ot[:])
                for kt in range(n_kt):
                    del hd["pt"][(qc, kt)]

            # ---- prologue: head 0 K-quant woven with its first QK quads ----
            alloc_head(0)
            emit_vext_const(0)
            emit_qchunk(0, 0)
            for g in range(n_grp):
                emit_kgroup(0, g)
                for kt in range(4 * g, 4 * g + 4):
                    emit_qk_one(0, 0, kt)

            # per-head preprocessing pieces for head h, woven into the 4
            # slots of head h-1 (piece index j = 0..3)
            def preprocess_pieces(h, j):
                if j == 0:
                    alloc_head(h)
                    emit_vext_const(h)
                    emit_qchunk(h, 0)
                    emit_kgroup(h, 0)
                    emit_kgroup(h, 1)
                elif j == 1:
                    for g in range(2, 6):
                        emit_kgroup(h, g)
                elif j == 2:
                    emit_kgroup(h, 6)
                    emit_kgroup(h, 7)
                    emit_qchunk(h, 1)
                    emit_qchunk(h, 2)
                    emit_vgroup(h, 0)
                    emit_vgroup(h, 1)
                else:
                    emit_qchunk(h, 3)
                    for g in range(2, 8):
                        emit_vgroup(h, g)

            # ---- main slot loop ----
            # slot s: QK(s) already handles s=0 (prologue); emit QK(s),
            # exps woven inside; PV(s-1); weave head(h+1) preprocessing and
            # head-0 leftovers (Q chunks 1-3, V groups) into early slots.
            for s in range(n_slots):
                h, qc = divmod(s, n_qc)
                if s > 0:
                    emit_qk(h, qc)
                # head-0 leftover staging woven into its own slots 0/1
                if s == 0:
                    emit_qchunk(0, 1)
                    for g in range(4):
                        emit_vgroup(0, g)
                elif s == 1:
                    emit_qchunk(0, 2)
                    emit_qchunk(0, 3)
                    for g in range(4, 8):
                        emit_vgroup(0, g)
                # next-head preprocessing pieces
                if h + 1 < H:
                    preprocess_pieces(h + 1, qc)
                # PV for previous slot
                if s > 0:
                    ph, pqc = divmod(s - 1, n_qc)
                    emit_pv(ph, pqc)
            emit_pv(H - 1, n_qc - 1)

    nc.compile()
    return nc


def get_nc(**kwargs):
    key = tuple(sorted(kwargs.items()))
    if key not in _NC_CACHE:
        _NC_CACHE[key] = _build_nc(**kwargs)
    return _NC_CACHE[key]


def kernel(q, k, v, _trace=False, _trace_cores=None, _nc_kwargs=None):
    """Full-input entry point: q/k/v [1, 4096, 16, 128] fp32 -> same shape."""
    assert q.shape == (1, _S, _H_TOTAL, _D), q.shape
    nc = get_nc(**(_nc_kwargs or {}))
    in_maps = []
    for c in range(_N_CORES):
        hs = slice(c * _H, (c + 1) * _H)
        in_maps.append({
            "qt": np.ascontiguousarray(
                q[0, :, hs, :].transpose(1, 2, 0), dtype=np.float32),
            "k": np.ascontiguousarray(
                k[0, :, hs, :].transpose(1, 0, 2), dtype=np.float32),
            "v": np.ascontiguousarray(
                v[0, :, hs, :].transpose(1, 0, 2), dtype=np.float32),
        })
    # The axon-tunneled device occasionally reports a transient
    # NRT_EXEC_UNIT_UNRECOVERABLE on the first execution; a retry succeeds.
    last_err = None
    for attempt in range(3):
        try:
            res = run_bass_kernel_spmd(nc, in_maps,
                                       core_ids=list(range(_N_CORES)),
                                       trace=_trace, trace_cores=_trace_cores)
            break
        except Exception as e:  # noqa: BLE001
            last_err = e
            time.sleep(2.0 * (attempt + 1))
    else:
        raise last_err
    out = np.stack([res.results[c]["o"] for c in range(_N_CORES)])
    out = np.ascontiguousarray(
        out.reshape(_H_TOTAL, _S, _D).transpose(1, 0, 2)[None],
        dtype=np.float32)
    if _trace:
        return out, res
    return out


# revision 16
# speedup vs baseline: 1.0700x; 1.0700x over previous
"""ChunkedAttention (nn_ChunkedAttention_43568148251092) Trainium2 kernel.

Full inputs q/k/v: [1, 4096, 16, 128] fp32. Shards the 16 heads across the
8 NeuronCores (2 heads per core, pure head parallelism - no collectives),
runs a Bass/Tile attention kernel per core, and reassembles the results.

Per-core pipeline (2 heads, S=4096 tokens, D=128). ACT-exp is the roofline
(2*S^2 exps at 1.2GHz*128 lanes ~= 267us busy); everything else is
organized to hide under it:
  - Host stages layouts only: Q fed pre-transposed [H, 128(d), 4096(s)],
    K/V per-head contiguous [H, 4096, 128], output per-head contiguous.
  - K/V int8 quant-dequant per token exactly like the reference
    (trunc-toward-zero), via a 5-op DVE chain: y = kv*(1/scale);
    y = (y + c) - (kv>0)*2c with c = 0.499995; RNE int32 convert.
    This equals trunc() except on a ~1e-5-wide window per integer boundary.
  - Kint (exact in fp16) is PE-transposed to KT [d, s]; per-token
    kscale/sqrt(D) folded into the softmax exp via the ACT per-partition
    scale. Q cast to fp16 on DVE in [d, s] chunks (no transposes).
  - S^T[k,q] = KT_tile.T @ QTchunk in PSUM fp32 (512-wide fp16 matmuls).
  - P'[k,q] = exp(kscale*S^T - 40) on ACT ([128,1024] tiles) -> bf16.
  - out[q, 0:128|denom] = sum_kt P'_kt.T @ [Vdq | ones], one contiguous
    32-matmul accumulation group per 128-q tile into a bank-aligned
    [128, 132] PSUM tile (PSUM groups must stay within one bank and not
    share zero regions); the ones-column gives the denominator for free.
  - Software pipelining: slot s = (head, q-chunk). The PE stream runs
    "4 QK tiles, then one 4-kt PV burst of slot s-1" so ACT always has
    scores to exp and P' buffers recycle at the production rate. Head h+1
    preprocessing is woven into head h's slots; head 0's first QK quads
    are woven into its K-quant groups so ACT starts ~8us into the kernel.
"""

import math
import time

import numpy as np

import concourse.bass as bass
import concourse.mybir as mybir
import concourse.tile as tile
from concourse import bacc
from concourse.bass_utils import run_bass_kernel_spmd
from concourse.masks import make_identity

F32 = mybir.dt.float32
BF16 = mybir.dt.bfloat16
FP16 = mybir.dt.float16
I32 = mybir.dt.int32
AX = mybir.AxisListType.X
OP = mybir.AluOpType
EXP = mybir.ActivationFunctionType.Exp

_S = 4096
_H_TOTAL = 16
_D = 128
_N_CORES = 8
_H = _H_TOTAL // _N_CORES  # heads per core

_NC_CACHE = {}

# trunc-toward-zero shift constant (see module docstring)
_C = 0.499995


def _bcast3(ap2, n):
    """[128, J] AP -> [128, J, n] broadcast AP (inner stride 0)."""
    return bass.AP(tensor=ap2.tensor, offset=ap2.offset, ap=[*ap2.ap, [0, n]])


def _build_nc(S=_S, H=_H, D=_D, qc_cols=1024, c_bias=40.0,
              pp_bufs=67, psS_bufs=2, ld_bufs=5, tmp_bufs=2):
    assert D == 128 and S % 512 == 0 and qc_cols == 1024
    n_kt = S // 128       # 32 K tiles per head
    n_grp = S // 512      # 8 quant groups per head
    n_qc = S // qc_cols   # 4 q chunks per head
    qt_per_qc = qc_cols // 128
    n_slots = H * n_qc

    nc = bacc.Bacc("TRN2")
    qt_d = nc.dram_tensor("qt", [H, D, S], F32, kind="ExternalInput")
    k_d = nc.dram_tensor("k", [H, S, D], F32, kind="ExternalInput")
    v_d = nc.dram_tensor("v", [H, S, D], F32, kind="ExternalInput")
    o_d = nc.dram_tensor("o", [H, S, D], F32, kind="ExternalOutput")

    with tile.TileContext(nc) as tc:
        with (
            tc.tile_pool(name="const", bufs=1) as constp,
            tc.tile_pool(name="big", bufs=2) as bigp,
            tc.tile_pool(name="qld", bufs=2) as qldp,
            tc.tile_pool(name="ld", bufs=ld_bufs) as ldp,
            tc.tile_pool(name="tmp", bufs=tmp_bufs) as tmpp,
            tc.tile_pool(name="b16", bufs=2) as b16p,
            tc.tile_pool(name="small", bufs=8) as smallp,
            tc.tile_pool(name="pp", bufs=pp_bufs) as ppool,
            tc.tile_pool(name="outp", bufs=4) as outp,
            tc.tile_pool(name="psS", bufs=psS_bufs, space="PSUM") as psS,
            tc.tile_pool(name="psT", bufs=2, space="PSUM") as psT,
            tc.tile_pool(name="psO", bufs=2, space="PSUM") as psO,
        ):
            ident32 = constp.tile([128, 128], F32)
            make_identity(nc, ident32[:])
            ident16 = constp.tile([128, 128], FP16)
            nc.vector.tensor_copy(ident16[:], ident32[:])
            bias_t = constp.tile([128, 1], F32)
            nc.vector.memset(bias_t[:], -c_bias)

            heads = [None] * H
            slot_state = {}

            def alloc_head(h):
                heads[h] = {
                    "KT": bigp.tile([128, S], FP16, tag="KT", name=f"KT{h}"),
                    "QTc": [None] * n_qc,
                    "Vext": bigp.tile([128, n_kt, 132], BF16, tag="Vext",
                                      name=f"Vext{h}"),
                    "ksc": bigp.tile([128, n_kt], F32, tag="ksc",
                                     name=f"ksc{h}"),
                    "pt": {},
                }

            def emit_vext_const(h):
                V = heads[h]["Vext"]
                nc.vector.memset(V[:, :, 128:132], 0.0)
                nc.vector.memset(V[:, :, 128:129], 1.0)

            def emit_qchunk(h, c):
                c0 = c * qc_cols
                qtc = bigp.tile([128, qc_cols], FP16, tag="QTc", bufs=4,
                                name=f"QTc{h}_{c}")
                for half in range(2):
                    qf = qldp.tile([128, 512], F32, tag="qld", name="qf")
                    nc.sync.dma_start(
                        out=qf[:],
                        in_=qt_d[h, :, c0 + half * 512:c0 + half * 512 + 512])
                    nc.vector.tensor_copy(
                        qtc[:, half * 512:half * 512 + 512], qf[:])
                heads[h]["QTc"][c] = qtc

            def quant_common(src_d, h, g):
                """Load + scale + trunc -> (i32 tile [128,4,128], sc [128,4])."""
                s0 = g * 512
                xf = ldp.tile([128, 4, 128], F32, tag="ld", name="xf")
                nc.sync.dma_start(
                    out=xf[:],
                    in_=src_d[h, s0:s0 + 512, :].rearrange(
                        "(j p) d -> p j d", p=128))
                am = smallp.tile([128, 4], F32, tag="am", name="am")
                nc.vector.reduce_max(am[:], xf[:], axis=AX,
                                     apply_absolute_value=True)
                sc = smallp.tile([128, 4], F32, tag="sc", name="sc")
                nc.vector.tensor_scalar(sc[:], am[:], 1e-8, 1.0 / 127.0,
                                        op0=OP.max, op1=OP.mult)
                rc = smallp.tile([128, 4], F32, tag="rc", name="rc")
                nc.vector.reciprocal(rc[:], sc[:])
                y = tmpp.tile([128, 4, 128], F32, tag="t_y", name="y")
                nc.vector.tensor_tensor(y[:], xf[:], _bcast3(rc[:], 128),
                                        op=OP.mult)
                g2c = tmpp.tile([128, 4, 128], F32, tag="t_g", name="g2c")
                nc.vector.tensor_scalar(g2c[:], xf[:], 0.0, 2.0 * _C,
                                        op0=OP.is_gt, op1=OP.mult)
                nc.vector.scalar_tensor_tensor(y[:], y[:], _C, g2c[:],
                                               op0=OP.add, op1=OP.subtract)
                xi = tmpp.tile([128, 4, 128], I32, tag="t_i", name="xi")
                nc.vector.tensor_copy(xi[:], y[:])
                return xi, sc

            def emit_kgroup(h, g):
                hd = heads[h]
                xi, sc = quant_common(k_d, h, g)
                nc.vector.tensor_scalar(hd["ksc"][:, 4 * g:4 * g + 4], sc[:],
                                        1.0 / math.sqrt(128.0), None,
                                        op0=OP.mult)
                kint = b16p.tile([128, 4, 128], FP16, tag="i16", name="kint")
                nc.vector.tensor_copy(kint[:], xi[:])
                for j in range(4):
                    pst = psT.tile([128, 128], FP16, tag="pst", name="pst")
                    nc.tensor.transpose(pst[:], kint[:, j, :], ident16[:])
                    kt_i = 4 * g + j
                    nc.vector.tensor_copy(
                        hd["KT"][:, kt_i * 128:(kt_i + 1) * 128], pst[:])

            def emit_vgroup(h, g):
                hd = heads[h]
                xi, sc = quant_common(v_d, h, g)
                vf = tmpp.tile([128, 4, 128], F32, tag="t_y", name="vf")
                nc.vector.tensor_copy(vf[:], xi[:])
                nc.vector.tensor_tensor(
                    hd["Vext"][:, 4 * g:4 * g + 4, 0:128], vf[:],
                    _bcast3(sc[:], 128), op=OP.mult)

            def emit_qk_one(h, qc, kt):
                hd = heads[h]
                sps = psS.tile([128, qc_cols], F32, tag="sps", name="sps")
                w = hd["KT"][:, kt * 128:(kt + 1) * 128]
                qtc = hd["QTc"][qc]
                for half in range(2):
                    c0 = half * 512
                    nc.tensor.matmul(sps[:, c0:c0 + 512],
                                     w, qtc[:, c0:c0 + 512],
                                     start=True, stop=True)
                pt = ppool.tile([128, qc_cols], BF16, tag="pp", name="pt")
                nc.scalar.activation(pt[:], sps[:], EXP, bias=bias_t[:],
                                     scale=hd["ksc"][:, kt:kt + 1])
                hd["pt"][(qc, kt)] = pt

            def emit_pv_block(s_prev, qt):
                """One q-tile of PV for slot s_prev: a contiguous 32-matmul
                accumulation group into its own bank-aligned PSUM tile,
                followed by the denominator divide + store."""
                h, qc = divmod(s_prev, n_qc)
                hd = heads[h]
                ops_ = psO.tile([128, 132], F32, tag="ops", name="ops")
                for kt in range(n_kt):
                    nc.tensor.matmul(
                        ops_[:],
                        hd["pt"][(qc, kt)][:, qt * 128:(qt + 1) * 128],
                        hd["Vext"][:, kt, :],
                        start=(kt == 0), stop=(kt == n_kt - 1))
                rcp = smallp.tile([128, 1], F32, tag="rcp", name="rcp")
                nc.vector.reciprocal(rcp[:], ops_[:, 128:129])
                ot = outp.tile([128, 128], F32, tag="ot", name="ot")
                nc.vector.tensor_scalar(ot[:], ops_[:, 0:128], rcp[:],
                                        None, op0=OP.mult)
                q0 = qc * qc_cols + qt * 128
                nc.sync.dma_start(out=o_d[h, q0:q0 + 128, :], in_=ot[:])
                if qt == qt_per_qc - 1:
                    for kt in range(n_kt):
                        del hd["pt"][(qc, kt)]

            # per-head preprocessing pieces for head h, woven into the 4
            # slots of head h-1 (piece index j = 0..3)
            def preprocess_pieces(h, j):
                if j == 0:
                    alloc_head(h)
                    emit_vext_const(h)
                    emit_qchunk(h, 0)
                    emit_kgroup(h, 0)
                    emit_kgroup(h, 1)
                elif j == 1:
                    for g in range(2, 6):
                        emit_kgroup(h, g)
                elif j == 2:
                    emit_kgroup(h, 6)
                    emit_kgroup(h, 7)
                    emit_vgroup(h, 0)
                    emit_vgroup(h, 1)
                else:
                    for g in range(2, 8):
                        emit_vgroup(h, g)

            # ---- prologue: head 0 K-quant woven with its first QK quads ----
            alloc_head(0)
            emit_vext_const(0)
            emit_qchunk(0, 0)
            for g in range(n_grp):
                emit_kgroup(0, g)
                for kt in range(4 * g, 4 * g + 4):
                    emit_qk_one(0, 0, kt)

            # ---- main slot loop ----
            for s in range(n_slots):
                h, qc = divmod(s, n_qc)
                # stage the Q chunk for this head's next slot
                if qc + 1 < n_qc:
                    emit_qchunk(h, qc + 1)
                if s == 0:
                    for g in range(4):
                        emit_vgroup(0, g)
                elif s == 1:
                    for g in range(4, 8):
                        emit_vgroup(0, g)
                if s > 0:
                    # QK for slot s with PV q-tile groups of slot s-1
                    # interleaved after every 4th QK tile
                    for kt in range(n_kt):
                        emit_qk_one(h, qc, kt)
                        if kt % 4 == 3:
                            emit_pv_block(s - 1, kt // 4)
                if h + 1 < H:
                    preprocess_pieces(h + 1, qc)
            # drain last slot's PV
            for qt in range(qt_per_qc):
                emit_pv_block(n_slots - 1, qt)

    nc.compile()
    return nc


def get_nc(**kwargs):
    key = tuple(sorted(kwargs.items()))
    if key not in _NC_CACHE:
        _NC_CACHE[key] = _build_nc(**kwargs)
    return _NC_CACHE[key]


def kernel(q, k, v, _trace=False, _trace_cores=None, _nc_kwargs=None):
    """Full-input entry point: q/k/v [1, 4096, 16, 128] fp32 -> same shape."""
    assert q.shape == (1, _S, _H_TOTAL, _D), q.shape
    nc = get_nc(**(_nc_kwargs or {}))
    in_maps = []
    for c in range(_N_CORES):
        hs = slice(c * _H, (c + 1) * _H)
        in_maps.append({
            "qt": np.ascontiguousarray(
                q[0, :, hs, :].transpose(1, 2, 0), dtype=np.float32),
            "k": np.ascontiguousarray(
                k[0, :, hs, :].transpose(1, 0, 2), dtype=np.float32),
            "v": np.ascontiguousarray(
                v[0, :, hs, :].transpose(1, 0, 2), dtype=np.float32),
        })
    # The axon-tunneled device occasionally reports a transient
    # NRT_EXEC_UNIT_UNRECOVERABLE on the first execution; a retry succeeds.
    last_err = None
    for attempt in range(3):
        try:
            res = run_bass_kernel_spmd(nc, in_maps,
                                       core_ids=list(range(_N_CORES)),
                                       trace=_trace, trace_cores=_trace_cores)
            break
        except Exception as e:  # noqa: BLE001
            last_err = e
            time.sleep(2.0 * (attempt + 1))
    else:
        raise last_err
    out = np.stack([res.results[c]["o"] for c in range(_N_CORES)])
    out = np.ascontiguousarray(
        out.reshape(_H_TOTAL, _S, _D).transpose(1, 0, 2)[None],
        dtype=np.float32)
    if _trace:
        return out, res
    return out
